# revision 32
# baseline (speedup 1.0000x reference)
"""CKGAT knowledge-GAT kernel for 8 Trainium2 NeuronCores (Bass/Tile).

Math (per batch element b, per side in {user, item}, per layer i):
  pi   = leaky_relu(nh.(W@a1) + g2r[nr] + nt.(W@a3), 0.2)   [B,T,N]
  att  = softmax_N(pi)
  nei  = sum_n att * E[nh]                                   [B,T,D]
  emb  = elu((nei + E[t]) @ W).sum(T)                        [B,D]
  e_u  = mean_T E[user_h0] + sum_i emb_u_i
  e_v  = E[items] + mean_T E[item_h0] + sum_i emb_v_i
  out  = sigmoid(sum_d e_u * e_v)

Sharding: data-parallel over B (64 per core); entity/relation tables and
W/a replicated; the g3 = E.(W@a3) table is precomputed sharded (12544
rows/core) and AllGathered.

Layout (per core): bt = b*32 + t in [0, 2048); partition p = bt//16,
btlow = bt%16. Neighbor slots are streamed so that gathered rows land at
[p, btlow*8+n] (dma_gather writes stream pos i to [i%128, i//128]).
Entity rows are fetched as 1024B 4-row blocks (int16 block ids) and the
right row is extracted with predicated copies; g3/g2r lookups use
select-trees over 256B blocks / an SBUF-resident 32-entry table.
"""

import numpy as np

P = 128
BC, T, NN, D = 64, 32, 8, 64
BT = BC * T  # 2048
NE, NR = 100000, 32
NCORES = 8
EPC = 12544            # padded entity rows per core (8*12544 = 100352)
NE_PAD = NCORES * EPC  # 100352
G3_ROWS = NE_PAD // 64  # 1568 blocks of 64 scalars

_CACHE = {}


def _build():
    import concourse.bass as bass
    import concourse.bacc as bacc
    import concourse.mybir as mybir
    from concourse.tile import TileContext
    from concourse.masks import make_identity

    fp32 = mybir.dt.float32
    i32 = mybir.dt.int32
    i16 = mybir.dt.int16
    Alu = mybir.AluOpType
    Act = mybir.ActivationFunctionType
    AxX = mybir.AxisListType.X

    def bc(ap_, *dims):
        """Append 0-stride broadcast dims to an AP."""
        return bass.AP(ap_.tensor, ap_.offset, list(ap_.ap) + [[0, d] for d in dims])

    def bcmid(t2d, n):
        """[128, M] tile -> [128, n(bcast), M] AP."""
        a = t2d[:]
        return bass.AP(a.tensor, a.offset, [list(a.ap[0]), [0, n], list(a.ap[1])])

    def dap(dram, dims, offset=0):
        """Arbitrary strided view of a DRAM tensor; dims = [(step, count), ...] els."""
        a = dram[:] if len(dram.shape) == 1 else dram[:, :]
        return bass.AP(a.tensor, a.offset + offset, [list(d) for d in dims])

    nc = bacc.Bacc("TRN2", target_bir_lowering=False, debug=False)

    ent = nc.dram_tensor("entity_emb", [NE, D], fp32, kind="ExternalInput")
    esl = nc.dram_tensor("e_slice", [EPC, D], fp32, kind="ExternalInput")
    rel = nc.dram_tensor("relation_emb", [NR, D], fp32, kind="ExternalInput")
    Wg = nc.dram_tensor("W_GAT", [D, D], fp32, kind="ExternalInput")
    ag = nc.dram_tensor("a_GAT", [3 * D, 1], fp32, kind="ExternalInput")
    items = nc.dram_tensor("items", [BC], i32, kind="ExternalInput")
    SIDES = ["u0", "u1", "i0", "i1"]
    nh_d = {s: nc.dram_tensor(f"nh_{s}", [BT, NN], i32, kind="ExternalInput") for s in SIDES}
    nr_d = {s: nc.dram_tensor(f"nr_{s}", [BT, NN], i32, kind="ExternalInput") for s in SIDES}
    nt_d = {s: nc.dram_tensor(f"nt_{s}", [BT, NN], i32, kind="ExternalInput") for s in SIDES}
    t_d = {s: nc.dram_tensor(f"t_{s}", [BT], i32, kind="ExternalInput") for s in SIDES}
    h0_d = {s: nc.dram_tensor(f"h0_{s}", [BT], i32, kind="ExternalInput") for s in ["u", "i"]}
    out_t = nc.dram_tensor("out", [1, BC], fp32, kind="ExternalOutput")

    g3in = nc.dram_tensor("g3in", [1, EPC], fp32)
    g3all = nc.dram_tensor("g3all", [G3_ROWS, 64], fp32)

    ent_blk = ent[:, :].rearrange("(a b) d -> a (b d)", b=4)  # [25000, 256]

    with TileContext(nc) as tc:
        with (
            tc.tile_pool(name="const", bufs=1) as cp,
            tc.tile_pool(name="side", bufs=2) as sp,
            tc.tile_pool(name="q", bufs=2) as qp,
            tc.tile_pool(name="psum", bufs=2, space="PSUM") as pp,
            tc.tile_pool(name="psum1", bufs=1, space="PSUM") as pp1,
        ):
            # ---------------- constants / precompute ----------------
            id128 = cp.tile([P, P], fp32)
            make_identity(nc, id128[:])

            Wt_s = cp.tile([D, D], fp32)
            nc.sync.dma_start(out=Wt_s[:], in_=Wg[:, :])
            a1_s = cp.tile([D, 1], fp32)
            nc.sync.dma_start(out=a1_s[:], in_=ag[0:D, :])
            a2_s = cp.tile([D, 1], fp32)
            nc.sync.dma_start(out=a2_s[:], in_=ag[D:2 * D, :])
            a3_s = cp.tile([D, 1], fp32)
            nc.sync.dma_start(out=a3_s[:], in_=ag[2 * D:3 * D, :])
            rel_s = cp.tile([NR, D], fp32)
            nc.sync.dma_start(out=rel_s[:], in_=rel[:, :])

            # W transposed (PE)
            WT_p = pp1.tile([D, D], fp32, space="PSUM", tag="pp1t")
            nc.tensor.transpose(out=WT_p[:], in_=Wt_s[:], identity=id128[0:D, 0:D])
            WT_s = cp.tile([D, D], fp32)
            nc.vector.tensor_copy(out=WT_s[:], in_=WT_p[:])

            # w1 = W @ a1, w3 = W @ a3 as [1, 64] rows
            w1_p = pp1.tile([1, D], fp32, space="PSUM", tag="pp1t")
            nc.tensor.matmul(out=w1_p[:], lhsT=a1_s[:], rhs=WT_s[:], start=True, stop=True)
            w1_s = cp.tile([1, D], fp32)
            nc.vector.tensor_copy(out=w1_s[:], in_=w1_p[:])
            w3_p = pp1.tile([1, D], fp32, space="PSUM", tag="pp1t")
            nc.tensor.matmul(out=w3_p[:], lhsT=a3_s[:], rhs=WT_s[:], start=True, stop=True)
            w3_s = cp.tile([1, D], fp32)
            nc.vector.tensor_copy(out=w3_s[:], in_=w3_p[:])

            # g2r[r] = (R @ W) . a2  -> [1, 32]
            RT_p = pp1.tile([D, NR], fp32, space="PSUM", tag="pp1t")
            nc.tensor.transpose(out=RT_p[:], in_=rel_s[:], identity=id128[0:NR, 0:NR])
            RT_s = cp.tile([D, NR], fp32)
            nc.vector.tensor_copy(out=RT_s[:], in_=RT_p[:])
            RWT_p = pp1.tile([D, NR], fp32, space="PSUM", tag="pp1t")
            nc.tensor.matmul(out=RWT_p[:], lhsT=Wt_s[:], rhs=RT_s[:], start=True, stop=True)
            RWT_s = cp.tile([D, NR], fp32)
            nc.vector.tensor_copy(out=RWT_s[:], in_=RWT_p[:])
            g2r_p = pp1.tile([1, NR], fp32, space="PSUM", tag="pp1t")
            nc.tensor.matmul(out=g2r_p[:], lhsT=a2_s[:], rhs=RWT_s[:], start=True, stop=True)
            g2r_s = cp.tile([1, NR], fp32)
            nc.vector.tensor_copy(out=g2r_s[:], in_=g2r_p[:])

            # replicate w1/w3/g2r across all 128 partitions (PE broadcast)
            ones1 = cp.tile([1, P], fp32)
            nc.gpsimd.memset(ones1[:], 1.0)
            w1b_p = pp1.tile([P, D], fp32, space="PSUM", tag="pp1t")
            nc.tensor.matmul(out=w1b_p[:], lhsT=ones1[:], rhs=w1_s[:], start=True, stop=True)
            w1b = cp.tile([P, D], fp32)
            nc.vector.tensor_copy(out=w1b[:], in_=w1b_p[:])
            w3b_p = pp1.tile([P, D], fp32, space="PSUM", tag="pp1t")
            nc.tensor.matmul(out=w3b_p[:], lhsT=ones1[:], rhs=w3_s[:], start=True, stop=True)
            w3b = cp.tile([P, D], fp32)
            nc.vector.tensor_copy(out=w3b[:], in_=w3b_p[:])
            g2rb_p = pp1.tile([P, NR], fp32, space="PSUM", tag="pp1t")
            nc.tensor.matmul(out=g2rb_p[:], lhsT=ones1[:], rhs=g2r_s[:], start=True, stop=True)
            g2rb = cp.tile([P, NR], fp32)
            nc.vector.tensor_copy(out=g2rb[:], in_=g2rb_p[:])

            # block-diag(W, W) for the (nei+t) @ W matmul on transposed chunks
            W2_s = cp.tile([P, P], fp32)
            nc.gpsimd.memset(W2_s[:], 0.0)
            nc.sync.dma_start(out=W2_s[0:D, 0:D], in_=Wg[:, :])
            nc.sync.dma_start(out=W2_s[D:P, D:P], in_=Wg[:, :])

            # stacked identity [[I],[I]] for summing partition halves via PE
            stack2 = cp.tile([P, D], fp32)
            nc.vector.tensor_copy(out=stack2[0:D, :], in_=id128[0:D, 0:D])
            nc.vector.tensor_copy(out=stack2[D:P, :], in_=id128[D:P, D:P])

            ones64 = cp.tile([D, 1], fp32)
            nc.gpsimd.memset(ones64[:], 1.0)

            # ---- g3 table: this core's 12544 rows of E . w3, then AllGather
            with tc.tile_pool(name="prep", bufs=1) as prp:
                g3part = prp.tile([P, 98], fp32)
                for j2 in range(7):
                    half = slice(14 * j2, 14 * j2 + 14)
                    echunk = prp.tile([P, 14 * D], fp32, tag="echunk", name="echunk")
                    nc.sync.dma_start(
                        out=echunk[:],
                        in_=dap(esl, [(98 * D, P), (1, 14 * D)], offset=j2 * 14 * D))
                    g3tmp = prp.tile([P, 14 * D], fp32, tag="g3tmp", name="g3tmp")
                    nc.vector.tensor_tensor(out=g3tmp[:].rearrange("p (j d) -> p j d", j=14),
                                            in0=echunk[:].rearrange("p (j d) -> p j d", j=14),
                                            in1=bcmid(w3b, 14),
                                            op=Alu.mult)
                    nc.vector.tensor_reduce(out=g3part[:, half],
                                            in_=g3tmp[:].rearrange("p (j d) -> p j d", j=14),
                                            axis=AxX, op=Alu.add)
                nc.sync.dma_start(out=g3in[0, :].rearrange("(p j) -> p j", p=P), in_=g3part[:])
                nc.gpsimd.collective_compute(
                    "AllGather", Alu.bypass,
                    ins=[g3in[:, :]],
                    outs=[g3all[:, :].rearrange("a b -> (a b)").rearrange("(c e) -> c e", c=NCORES)],
                    replica_groups=[list(range(NCORES))],
                )

            # e_u / e_v accumulators [64 dout, 64 b]
            e_acc = {}
            for k in ["u", "v"]:
                e_acc[k] = cp.tile([P, BC], fp32, tag=f"eacc_{k}", name=f"eacc_{k}")
                nc.gpsimd.memset(e_acc[k][:], 0.0)

            # ---------------- helpers ----------------
            def build_stream_n(dram, shift):
                """[BT, 8] int32 indices -> replicated int16 stream tile [128, 1024],
                stream pos i = (btlow*8+n)*128 + p, value = idx >> shift."""
                l3 = sp.tile([16, 1024], i32, tag="l3")
                # L3[q, (w, l, n)] = idx[bt=(w*16+q)*16+l, n]
                nc.sync.dma_start(out=l3[:].rearrange("q (w l n) -> q w l n", w=8, l=16),
                                  in_=dap(dram, [(128, 16), (2048, 8), (8, 16), (1, 8)]))
                s32 = sp.tile([16, 1024], i32, tag="s32")
                nc.scalar.copy(s32[:].rearrange("q (x w) -> q x w", w=8),
                               l3[:].rearrange("q (w x) -> q x w", w=8))
                nc.vector.tensor_scalar(out=s32[:], in0=s32[:], scalar1=shift, scalar2=None,
                                        op0=Alu.logical_shift_right)
                s16 = sp.tile([16, 1024], i16, tag="s16")
                nc.vector.tensor_copy(out=s16[:], in_=s32[:])
                full = sp.tile([P, 1024], i16, tag=f"sf_{shift}")
                for k in range(8):
                    nc.sync.dma_start(out=full[16 * k:16 * k + 16, :], in_=s16[:])
                return full

            def build_stream_t(dram, ncols=128, wq=8):
                """[BT] int32 -> replicated int16 stream [128, ncols], pos i = btlow*128+p,
                value = idx >> 2 (row-block ids)."""
                l3 = sp.tile([16, ncols], i32, tag="l3t")
                # L3[q, (w, l)] = idx[bt=(w*16+q)*16+l]
                nc.sync.dma_start(out=l3[:].rearrange("q (w l) -> q w l", w=wq),
                                  in_=dap(dram, [(16, 16), (256, wq), (1, 16)]))
                s32 = sp.tile([16, ncols], i32, tag="s32t")
                nc.scalar.copy(s32[:].rearrange("q (x w) -> q x w", w=wq),
                               l3[:].rearrange("q (w x) -> q x w", w=wq))
                nc.vector.tensor_scalar(out=s32[:], in0=s32[:], scalar1=2, scalar2=None,
                                        op0=Alu.logical_shift_right)
                s16 = sp.tile([16, ncols], i16, tag="s16t")
                nc.vector.tensor_copy(out=s16[:], in_=s32[:])
                full = sp.tile([P, ncols], i16, tag="sft")
                for k in range(8):
                    nc.sync.dma_start(out=full[16 * k:16 * k + 16, :], in_=s16[:])
                return full

            def rmask3(nat32, tag):
                """sub-row masks (idx&3)==k for k=1,2,3 from a natural-layout int32 tile."""
                rr = sp.tile(list(nat32.shape), i32, tag=f"rr_{tag}")
                nc.vector.tensor_scalar(out=rr[:], in0=nat32[:], scalar1=3, scalar2=None,
                                        op0=Alu.bitwise_and)
                ms = []
                for k in (1, 2, 3):
                    m = sp.tile(list(nat32.shape), i32, tag=f"rm{k}_{tag}")
                    nc.vector.tensor_scalar(out=m[:], in0=rr[:], scalar1=k, scalar2=None,
                                            op0=Alu.is_equal)
                    ms.append(m)
                return ms

            def extract_rows(gblk, masks, mslice, nslots, tag):
                """gblk [128, nslots, 4, 64] -> rows [128, nslots, 64] using per-slot
                sub-row masks (mslice(m) is the [128, nslots] view of each mask)."""
                c = qp.tile([P, nslots * D], fp32, tag=f"C_{tag}")
                cv = c[:].rearrange("p (s d) -> p s d", s=nslots)
                gv = gblk[:].rearrange("p (s r d) -> p s r d", s=nslots, r=4)
                nc.scalar.copy(cv, gv[:, :, 0, :])
                for k in (1, 2, 3):
                    nc.vector.copy_predicated(out=cv, mask=bc(mslice(masks[k - 1]), D),
                                              data=gv[:, :, k, :])
                return c

            def bit_masks(nat32, bits, tag):
                ms = []
                for b in bits:
                    m = sp.tile(list(nat32.shape), i32, tag=f"bm{b}_{tag}")
                    nc.vector.tensor_scalar(out=m[:], in0=nat32[:], scalar1=b, scalar2=None,
                                            op0=Alu.bitwise_and)
                    ms.append(m)
                return ms

            # ---------------- per-side processing ----------------
            for s in SIDES:
                acc = e_acc["u" if s[0] == "u" else "v"]

                nat_nh = sp.tile([P, P], i32, tag="natnh")
                nc.sync.dma_start(out=nat_nh[:], in_=nh_d[s][:, :].rearrange("(p l) n -> p (l n)", l=16))
                nat_nt = sp.tile([P, P], i32, tag="natnt")
                nc.sync.dma_start(out=nat_nt[:], in_=nt_d[s][:, :].rearrange("(p l) n -> p (l n)", l=16))
                nat_nr = sp.tile([P, P], i32, tag="natnr")
                nc.sync.dma_start(out=nat_nr[:], in_=nr_d[s][:, :].rearrange("(p l) n -> p (l n)", l=16))
                nat_t = sp.tile([P, 16], i32, tag="natt")
                nc.sync.dma_start(out=nat_t[:], in_=t_d[s][:].rearrange("(p l) -> p l", l=16))

                s_nh = build_stream_n(nh_d[s], 2)        # row-block ids of nh
                # g3 block stream: (nt >> 6)
                s_nt6 = build_stream_n(nt_d[s], 6)
                s_t = build_stream_t(t_d[s])

                nh_rm = rmask3(nat_nh, "nh")             # [128,128] each
                t_rm = rmask3(nat_t, "t")                # [128,16]
                nt_sub = sp.tile([P, P], i32, tag="ntsub")
                nc.vector.tensor_scalar(out=nt_sub[:], in0=nat_nt[:], scalar1=63, scalar2=None,
                                        op0=Alu.bitwise_and)
                nt_bits = bit_masks(nt_sub, [32, 16, 8, 4, 2, 1], "nt")
                nr_bits = bit_masks(nat_nr, [16, 8, 4, 2, 1], "nr")

                # g2r lookup for the whole side: [128, 128]
                rin = None
                width = 16
                lvl = 0
                while width >= 1:
                    rt = sp.tile([P, P * width], fp32, tag=f"t2s_{width}", name=f"t2s_{width}", bufs=1)
                    rv = rt[:].rearrange("p (s e) -> p s e", s=P)
                    if lvl == 0:
                        a0 = g2rb[:]
                        lo = bass.AP(a0.tensor, a0.offset, [list(a0.ap[0]), [0, P], [1, 16]])
                        hi = bass.AP(a0.tensor, a0.offset + 16, [list(a0.ap[0]), [0, P], [1, 16]])
                    else:
                        lo = rin[:, :, 0:width]
                        hi = rin[:, :, width:2 * width]
                    nc.scalar.copy(rv, lo)
                    nc.vector.copy_predicated(out=rv, mask=bc(nr_bits[lvl][:, :], width), data=hi)
                    rin = rv
                    width //= 2
                    lvl += 1
                g2side = rin  # [128, 128, 1]

                # t rows for the whole side: [128, 16, 64]
                gt = sp.tile([P, 16 * 4 * D], fp32, tag="gt")
                nc.gpsimd.dma_gather(
                    out_ap=gt[:].rearrange("p (k e) -> p k e", k=16),
                    in_ap=ent_blk, idxs_ap=s_t[:], num_idxs=BT, num_idxs_reg=BT,
                    elem_size=256, single_packet=False)
                trows = qp.tile([P, 16 * D], fp32, tag="trows")
                tv = trows[:].rearrange("p (s d) -> p s d", s=16)
                gtv = gt[:].rearrange("p (s r d) -> p s r d", s=16, r=4)
                nc.scalar.copy(tv, gtv[:, :, 0, :])
                for k in (1, 2, 3):
                    nc.vector.copy_predicated(out=tv, mask=bc(t_rm[k - 1][:], D),
                                              data=gtv[:, :, k, :])

                for q in range(8):
                    qs = slice(16 * q, 16 * q + 16)      # natural-layout col slice
                    # gathers
                    g = qp.tile([P, 16 * 4 * D], fp32, tag="G")
                    nc.gpsimd.dma_gather(
                        out_ap=g[:].rearrange("p (k e) -> p k e", k=16),
                        in_ap=ent_blk, idxs_ap=s_nh[:, 128 * q:128 * q + 128],
                        num_idxs=2048, num_idxs_reg=2048, elem_size=256, single_packet=False)
                    g3 = qp.tile([P, 16 * 64], fp32, tag="G3")
                    nc.gpsimd.dma_gather(
                        out_ap=g3[:].rearrange("p (k e) -> p k e", k=16),
                        in_ap=g3all[:, :], idxs_ap=s_nt6[:, 128 * q:128 * q + 128],
                        num_idxs=2048, num_idxs_reg=2048, elem_size=64, single_packet=False)

                    # nh row extraction
                    C = extract_rows(g, nh_rm, lambda m: m[:, qs], 16, "nh")
                    Cv = C[:].rearrange("p (s d) -> p s d", s=16)

                    # s1 = C . w1
                    tmp1 = qp.tile([P, 16 * D], fp32, tag="wtmp")
                    nc.vector.tensor_tensor(
                        out=tmp1[:].rearrange("p (s d) -> p s d", s=16), in0=Cv,
                        in1=bcmid(w1b, 16),
                        op=Alu.mult)
                    s1 = qp.tile([P, 16], fp32, tag="s1")
                    nc.vector.tensor_reduce(out=s1[:], in_=tmp1[:].rearrange("p (s d) -> p s d", s=16),
                                            axis=AxX, op=Alu.add)

                    # g3 select tree: [128, 16, 64] -> [128, 16]
                    tin = g3[:].rearrange("p (s e) -> p s e", s=16)
                    width = 32
                    lvl = 0
                    while width >= 1:
                        tt = qp.tile([P, 16 * width], fp32, tag=f"t3_{width}", name=f"t3_{width}")
                        tv3 = tt[:].rearrange("p (s e) -> p s e", s=16)
                        nc.scalar.copy(tv3, tin[:, :, 0:width])
                        nc.vector.copy_predicated(
                            out=tv3, mask=bc(nt_bits[lvl][:, qs], width),
                            data=tin[:, :, width:2 * width])
                        tin = tv3
                        width //= 2
                        lvl += 1
                    g3v = tin  # [128, 16, 1]

                    g2v = g2side[:, qs, :]

                    # pi, leaky relu, softmax over n
                    pi = qp.tile([P, 16], fp32, tag="pi")
                    nc.vector.tensor_tensor(out=pi[:], in0=s1[:],
                                            in1=g3v[:, :, 0], op=Alu.add)
                    nc.vector.tensor_tensor(out=pi[:], in0=pi[:],
                                            in1=g2v[:, :, 0], op=Alu.add)
                    piL = qp.tile([P, 16], fp32, tag="piL")
                    nc.vector.tensor_scalar(out=piL[:], in0=pi[:], scalar1=0.2, scalar2=None,
                                            op0=Alu.mult)
                    nc.vector.tensor_tensor(out=piL[:], in0=piL[:], in1=pi[:], op=Alu.max)
                    ex = qp.tile([P, 16], fp32, tag="ex")
                    nc.scalar.activation(ex[:], piL[:], Act.Exp)
                    den = qp.tile([P, 2], fp32, tag="den")
                    nc.vector.tensor_reduce(out=den[:], in_=ex[:].rearrange("p (l n) -> p l n", l=2),
                                            axis=AxX, op=Alu.add)
                    rinv = qp.tile([P, 2], fp32, tag="rinv")
                    nc.vector.reciprocal(out=rinv[:], in_=den[:])
                    att = qp.tile([P, 16], fp32, tag="att")
                    nc.vector.tensor_tensor(
                        out=att[:].rearrange("p (l n) -> p l n", l=2),
                        in0=ex[:].rearrange("p (l n) -> p l n", l=2),
                        in1=bass.AP(rinv[:].tensor, rinv[:].offset, [list(rinv[:].ap[0]), [1, 2], [0, 8]]),
                        op=Alu.mult)

                    # nei = sum_n att * C
                    wtmp = qp.tile([P, 16 * D], fp32, tag="wtmp")
                    nc.vector.tensor_tensor(
                        out=wtmp[:].rearrange("p (l n d) -> p l n d", l=2, n=8),
                        in0=C[:].rearrange("p (l n d) -> p l n d", l=2, n=8),
                        in1=bass.AP(att[:].tensor, att[:].offset,
                                    [list(att[:].ap[0]), [8, 2], [1, 8], [0, D]]),
                        op=Alu.mult)
                    X = qp.tile([P, 2 * D], fp32, tag="X")
                    nc.vector.tensor_reduce(
                        out=X[:].rearrange("p (l d) -> p l d", l=2),
                        in_=bass.AP(wtmp[:].tensor, wtmp[:].offset,
                                    [list(wtmp[:].ap[0]), [512, 2], [1, D], [D, 8]]),
                        axis=AxX, op=Alu.add)
                    # X += t rows
                    nc.vector.tensor_tensor(out=X[:], in0=X[:],
                                            in1=trows[:, 128 * q:128 * q + 128], op=Alu.add)

                    # (X @ W) with elu, T-sum into acc
                    xt_p = pp.tile([P, P], fp32, space="PSUM", tag="xt")
                    nc.tensor.transpose(out=xt_p[:], in_=X[:], identity=id128[:])
                    xt_s = qp.tile([P, P], fp32, tag="xts")
                    nc.scalar.copy(xt_s[:], xt_p[:])
                    y_p = pp.tile([P, P], fp32, space="PSUM", tag="y")
                    nc.tensor.matmul(out=y_p[:], lhsT=W2_s[:], rhs=xt_s[:], start=True, stop=True)
                    e1 = qp.tile([P, P], fp32, tag="e1")
                    nc.scalar.activation(e1[:], y_p[:], Act.Exp)
                    r1 = qp.tile([P, P], fp32, tag="r1")
                    nc.scalar.activation(r1[:], y_p[:], Act.Relu)
                    nc.vector.tensor_scalar(out=e1[:], in0=e1[:], scalar1=1.0, scalar2=None,
                                            op0=Alu.min)
                    nc.vector.tensor_tensor(out=e1[:], in0=e1[:], in1=r1[:], op=Alu.add)
                    ev1 = e1[:].rearrange("p (b two) -> p b two", two=2)
                    nc.vector.tensor_tensor(out=acc[:], in0=acc[:], in1=ev1[:, :, 0], op=Alu.add)
                    nc.vector.tensor_tensor(out=acc[:], in0=acc[:], in1=ev1[:, :, 1], op=Alu.add)

            # ---------------- layer-0 terms ----------------
            # mean_T E[user_h0] -> e_u ; mean_T E[item_h0] -> e_v
            for hs, k in [("u", "u"), ("i", "v")]:
                s_h = build_stream_t(h0_d[hs])
                nat_h = sp.tile([P, 16], i32, tag="nath")
                nc.sync.dma_start(out=nat_h[:], in_=h0_d[hs][:].rearrange("(p l) -> p l", l=16))
                h_rm = rmask3(nat_h, "h0")
                gh = sp.tile([P, 16 * 4 * D], fp32, tag="gt")
                nc.gpsimd.dma_gather(
                    out_ap=gh[:].rearrange("p (kk e) -> p kk e", kk=16),
                    in_ap=ent_blk, idxs_ap=s_h[:], num_idxs=BT, num_idxs_reg=BT,
                    elem_size=256, single_packet=False)
                hrows = qp.tile([P, 16 * D], fp32, tag="hrows")
                hv = hrows[:].rearrange("p (s d) -> p s d", s=16)
                ghv = gh[:].rearrange("p (s r d) -> p s r d", s=16, r=4)
                nc.scalar.copy(hv, ghv[:, :, 0, :])
                for kk in (1, 2, 3):
                    nc.vector.copy_predicated(out=hv, mask=bc(h_rm[kk - 1][:], D),
                                              data=ghv[:, :, kk, :])
                nc.vector.tensor_scalar(out=hrows[:], in0=hrows[:], scalar1=1.0 / T,
                                        scalar2=None, op0=Alu.mult)
                for ch in range(8):
                    ht_p = pp.tile([P, P], fp32, space="PSUM", tag="ht")
                    nc.tensor.transpose(out=ht_p[:], in_=hrows[:, 128 * ch:128 * ch + 128],
                                        identity=id128[:])
                    ht_s = qp.tile([P, P], fp32, tag="hts")
                    nc.scalar.copy(ht_s[:], ht_p[:])
                    hsum = qp.tile([P, D], fp32, tag="hsum")
                    nc.vector.tensor_reduce(
                        out=hsum[:], in_=ht_s[:].rearrange("p (b two) -> p b two", two=2),
                        axis=AxX, op=Alu.add)
                    nc.vector.tensor_tensor(out=e_acc[k][:], in0=e_acc[k][:], in1=hsum[:], op=Alu.add)

            # E[items] -> e_v
            s_it = sp.tile([16, 4], i32, tag="sit32")
            nc.sync.dma_start(out=s_it[:], in_=items[:].rearrange("(w q) -> q w", w=4))
            nc.vector.tensor_scalar(out=s_it[:], in0=s_it[:], scalar1=2, scalar2=None,
                                    op0=Alu.logical_shift_right)
            s_it16 = sp.tile([16, 4], i16, tag="sit16")
            nc.vector.tensor_copy(out=s_it16[:], in_=s_it[:])
            s_itf = sp.tile([P, 4], i16, tag="sitf")
            for k in range(8):
                nc.sync.dma_start(out=s_itf[16 * k:16 * k + 16, :], in_=s_it16[:])
            nat_it = sp.tile([BC, 1], i32, tag="natit")
            nc.sync.dma_start(out=nat_it[:], in_=items[:, None])
            it_rm = rmask3(nat_it, "it")
            git = sp.tile([P, 4 * D], fp32, tag="git")
            nc.gpsimd.dma_gather(
                out_ap=git[:].rearrange("p (kk e) -> p kk e", kk=1),
                in_ap=ent_blk, idxs_ap=s_itf[:], num_idxs=BC, num_idxs_reg=BC,
                elem_size=256, single_packet=False)
            itrows = qp.tile([BC, D], fp32, tag="itrows")
            gitv = git[0:BC, :].rearrange("p (s r d) -> p s r d", s=1, r=4)
            nc.vector.tensor_copy(out=itrows[:].rearrange("p (s d) -> p s d", s=1), in_=gitv[:, :, 0, :])
            for kk in (1, 2, 3):
                nc.vector.copy_predicated(out=itrows[:].rearrange("p (s d) -> p s d", s=1),
                                          mask=bc(it_rm[kk - 1][:], D), data=gitv[:, :, kk, :])
            it_p = pp.tile([D, BC], fp32, space="PSUM", tag="ht")
            nc.tensor.transpose(out=it_p[:], in_=itrows[:], identity=id128[0:BC, 0:BC])
            nc.vector.tensor_tensor(out=e_acc["v"][0:D, :], in0=e_acc["v"][0:D, :], in1=it_p[:], op=Alu.add)

            # ---------------- final: sigmoid(e_u . e_v) ----------------
            eu_p = pp1.tile([D, BC], fp32, space="PSUM", tag="pp1t")
            nc.tensor.matmul(out=eu_p[:], lhsT=stack2[:], rhs=e_acc["u"][:], start=True, stop=True)
            eu_s = cp.tile([D, BC], fp32)
            nc.vector.tensor_scalar(out=eu_s[:], in0=eu_p[:], scalar1=float(2 * T),
                                    scalar2=None, op0=Alu.subtract)
            ev_p = pp1.tile([D, BC], fp32, space="PSUM", tag="pp1t")
            nc.tensor.matmul(out=ev_p[:], lhsT=stack2[:], rhs=e_acc["v"][:], start=True, stop=True)
            ev_s = cp.tile([D, BC], fp32)
            nc.vector.tensor_scalar(out=ev_s[:], in0=ev_p[:], scalar1=float(2 * T),
                                    scalar2=None, op0=Alu.subtract)
            prod = cp.tile([D, BC], fp32)
            nc.vector.tensor_tensor(out=prod[:], in0=eu_s[:], in1=ev_s[:], op=Alu.mult)
            dot_p = pp1.tile([1, BC], fp32, space="PSUM", tag="pp1t")
            nc.tensor.matmul(out=dot_p[:], lhsT=ones64[:], rhs=prod[:], start=True, stop=True)
            sig = cp.tile([1, BC], fp32)
            nc.scalar.activation(sig[:], dot_p[:], Act.Sigmoid)
            nc.sync.dma_start(out=out_t[:, :], in_=sig[:])

    nc.compile()
    return nc


def _prep_inputs(inputs):
    """Build the 8 per-core input maps from full inputs."""
    f32 = np.float32
    ent = np.ascontiguousarray(np.asarray(inputs["entity_emb"], f32))
    rel = np.ascontiguousarray(np.asarray(inputs["relation_emb"], f32))
    Wg = np.ascontiguousarray(np.asarray(inputs["W_GAT"], f32))
    ag = np.ascontiguousarray(np.asarray(inputs["a_GAT"], f32))
    ent_pad = np.zeros((NE_PAD, D), f32)
    ent_pad[:NE] = ent

    def i32(x):
        return np.ascontiguousarray(np.asarray(x, np.int32))

    items = i32(inputs["items"])
    uh, ur, ut = i32(inputs["user_h"]), i32(inputs["user_r"]), i32(inputs["user_t"])
    unh, unr, unt = i32(inputs["user_nh"]), i32(inputs["user_nr"]), i32(inputs["user_nt"])
    ih, ir, it_ = i32(inputs["item_h"]), i32(inputs["item_r"]), i32(inputs["item_t"])
    inh, inr, int_ = i32(inputs["item_nh"]), i32(inputs["item_nr"]), i32(inputs["item_nt"])

    maps = []
    for c in range(NCORES):
        bs = slice(c * BC, (c + 1) * BC)
        m = {
            "entity_emb": ent,
            "e_slice": ent_pad[c * EPC:(c + 1) * EPC],
            "relation_emb": rel,
            "W_GAT": Wg,
            "a_GAT": ag,
            "items": items[bs],
            "h0_u": uh[0, bs].reshape(BT),
            "h0_i": ih[0, bs].reshape(BT),
        }
        for li in range(2):
            m[f"nh_u{li}"] = unh[li, bs].reshape(BT, NN)
            m[f"nr_u{li}"] = unr[li, bs].reshape(BT, NN)
            m[f"nt_u{li}"] = unt[li, bs].reshape(BT, NN)
            m[f"t_u{li}"] = ut[li, bs].reshape(BT)
            m[f"nh_i{li}"] = inh[li, bs].reshape(BT, NN)
            m[f"nr_i{li}"] = inr[li, bs].reshape(BT, NN)
            m[f"nt_i{li}"] = int_[li, bs].reshape(BT, NN)
            m[f"t_i{li}"] = it_[li, bs].reshape(BT)
        maps.append(m)
    return maps


def kernel(**inputs) -> np.ndarray:
    from concourse import bass_utils
    if "nc" not in _CACHE:
        _CACHE["nc"] = _build()
    nc = _CACHE["nc"]
    maps = _prep_inputs(inputs)
    res = bass_utils.run_bass_kernel_spmd(nc, maps, core_ids=list(range(NCORES)))
    return np.concatenate([res.results[c]["out"][0] for c in range(NCORES)]).astype(np.float32)
